# revision 3
# baseline (speedup 1.0000x reference)
"""Trainium2 Bass kernel for nn_Codec (causal conv codec + histogram entropy).

Sharding: the 12 channel-images (4*3 x 512x512) are split into 48
channel-pure slices of 128 rows; core k owns slices [6k, 6k+6).
Each core runs the 3-predictor conv pipeline on its slices and emits
partial sum-of-squares plus exact per-slice 256-bin histogram counts
(hi/lo 16x16 outer products accumulated on the PE).  The host sums the
partials across cores (the final all-reduce) and applies the scalar
epilogue (sqrt / entropy over the 256 bins).
"""

import numpy as np
import ml_dtypes

import concourse.bass as bass
import concourse.bacc as bacc
import concourse.tile as tile
from concourse import mybir
from concourse.bass_utils import run_bass_kernel_spmd

F32 = mybir.dt.float32
F32R = mybir.dt.float32r
BF16 = mybir.dt.bfloat16
ALU = mybir.AluOpType
ACTF = mybir.ActivationFunctionType

NSLICE = 6
ROWS = 128
W = 512
WP = 520
NEG = 0.01

_CACHE = {}
_LAST_RUN = None


def _build_weight_arrays(inp):
    w0bd = np.zeros((3, 128, 128), np.float32)
    w1bd = np.zeros((3, 128, 128), np.float32)
    w2bd = np.zeros((3, 128, 128), np.float32)
    w3z = np.zeros((3, 32, 128, 128), np.float32)
    biases = np.zeros((128, 9), np.float32)
    b3 = np.zeros(3, np.float32)
    for pi, p in enumerate("abc"):
        wT = np.asarray(inp[p + "_wT"], np.float32)
        wL = np.asarray(inp[p + "_wL"], np.float32)
        w1 = np.asarray(inp[p + "_w1"], np.float32)[:, :, 0, 0]
        w2 = np.asarray(inp[p + "_w2"], np.float32)[:, :, 0, 0]
        w3 = np.asarray(inp[p + "_w3"], np.float32)[0, :, 0, 0]
        tap = np.zeros((32, 32), np.float32)  # [t=8q+d, out_ch]
        for q in range(3):
            for d in range(7):
                tap[8 * q + d, :] = wT[:, 0, q, d]
        for d in range(3):
            tap[24 + d, :] = wL[:, 0, 0, d]
        for g in range(4):
            s = 32 * g
            w0bd[pi, s:s + 32, s:s + 32] = tap
            w1bd[pi, s:s + 32, s:s + 32] = w1.T
            w2bd[pi, s:s + 32, s:s + 32] = w2.T
            for t in range(32):
                w3z[pi, t, s:s + 32, 4 * t + g] = w3
        for l, b in enumerate([inp[p + "_bT"], inp[p + "_b1"], inp[p + "_b2"]]):
            biases[:, 3 * l + pi] = np.tile(np.asarray(b, np.float32), 4)
        b3[pi] = float(np.asarray(inp[p + "_b3"])[0])
    # iotaA: col 8i+s -> i+8 (hi' compare); iotaB: col 8i+s -> i (lo compare)
    ii = (np.arange(128) // 8).astype(np.float32)
    iotaA = np.tile(ii + 8.0, (128, 1)).astype(np.float32)
    iotaB = np.tile(ii, (128, 1)).astype(np.float32)
    return w0bd, w1bd, w2bd, w3z, biases, b3, iotaA, iotaB


def _build_bass(b3):
    nc = bacc.Bacc("TRN2", target_bir_lowering=False, debug=False, enable_asserts=False)
    xpad = nc.dram_tensor("xpad", (NSLICE * (ROWS + 3) * WP,), BF16, kind="ExternalInput")
    xf32 = nc.dram_tensor("xf32", (NSLICE * 128 * 512,), F32, kind="ExternalInput")
    w0 = nc.dram_tensor("w0", (3 * 128, 128), BF16, kind="ExternalInput")
    w1 = nc.dram_tensor("w1", (3 * 128, 128), BF16, kind="ExternalInput")
    w2 = nc.dram_tensor("w2", (3 * 128, 128), BF16, kind="ExternalInput")
    w3 = nc.dram_tensor("w3", (96 * 128, 128), BF16, kind="ExternalInput")
    bias_in = nc.dram_tensor("bias_in", (128, 9), F32, kind="ExternalInput")
    iotaA_in = nc.dram_tensor("iotaA_in", (128, 128), F32, kind="ExternalInput")
    iotaB_in = nc.dram_tensor("iotaB_in", (128, 128), F32, kind="ExternalInput")
    hist_out = nc.dram_tensor("hist_out", (NSLICE * 128, 256), F32, kind="ExternalOutput")
    ssq_out = nc.dram_tensor("ssq_out", (128, 2 * NSLICE), F32, kind="ExternalOutput")
    dma = nc.default_dma_engine

    with tile.TileContext(nc) as tc:
        with (
            tc.tile_pool(name="const", bufs=1) as constp,
            tc.tile_pool(name="taps", bufs=3) as tapp,
            tc.tile_pool(name="acts", bufs=3) as actp,
            tc.tile_pool(name="tail", bufs=2) as tailp,
            tc.tile_pool(name="oh", bufs=1) as ohp,
            tc.tile_pool(name="psA", bufs=2, space="PSUM") as psA,
            tc.tile_pool(name="psL3", bufs=3, space="PSUM") as psL3,
            tc.tile_pool(name="psH", bufs=1, space="PSUM") as psH,
        ):
            w0t = constp.tile([128, 3 * 128], BF16)
            w1t = constp.tile([128, 3 * 128], BF16)
            w2t = constp.tile([128, 3 * 128], BF16)
            w3t = constp.tile([128, 96 * 128], BF16)
            biast = constp.tile([128, 9], F32)
            iotaAt = constp.tile([128, 128], BF16)
            iotaBt = constp.tile([128, 128], BF16)
            tmpio = constp.tile([128, 128], F32)
            for k in range(3):
                dma.dma_start(out=w0t[:, 128 * k:128 * (k + 1)], in_=w0[128 * k:128 * (k + 1), :])
                dma.dma_start(out=w1t[:, 128 * k:128 * (k + 1)], in_=w1[128 * k:128 * (k + 1), :])
                dma.dma_start(out=w2t[:, 128 * k:128 * (k + 1)], in_=w2[128 * k:128 * (k + 1), :])
            for k in range(96):
                dma.dma_start(out=w3t[:, 128 * k:128 * (k + 1)], in_=w3[128 * k:128 * (k + 1), :])
            dma.dma_start(out=biast, in_=bias_in[:, :])
            dma.dma_start(out=tmpio, in_=iotaA_in[:, :])
            nc.vector.tensor_copy(out=iotaAt, in_=tmpio)
            tmpio2 = constp.tile([128, 128], F32)
            dma.dma_start(out=tmpio2, in_=iotaB_in[:, :])
            nc.vector.tensor_copy(out=iotaBt, in_=tmpio2)
            ssq_acc = constp.tile([128, 2 * NSLICE], F32)
            nc.vector.memset(ssq_acc[:, :], 0.0)

            drain_ct = 0
            for sl in range(NSLICE):
                base = sl * (ROWS + 3) * WP
                l3banks = []
                for _pi in range(3):
                    l3b = psL3.tile([128, 512], F32, tag="l3", name="l3b%d_%d" % (sl, _pi))
                    l3banks.append(l3b)
                for b8 in range(8):
                    for up in range(2):  # pairs of row-quads
                        tapts = []
                        for ui in range(2):
                            u = 2 * up + ui
                            r0 = 16 * b8 + 4 * u
                            tapt = tapp.tile([128, 512], BF16, tag="tap")
                            for s in range(4):
                                src = bass.AP(
                                    tensor=xpad,
                                    offset=base + (r0 + s) * WP,
                                    ap=[[WP, 4], [1, 8], [1, 512]],
                                )
                                dma.dma_start(out=tapt[32 * s:32 * (s + 1), :], in_=src)
                            tapts.append(tapt)
                        for pi in range(3):
                            cur = tapts
                            wts = [w0t, w1t, w2t]
                            for layer in range(3):
                                bank = psA.tile([128, 1024], F32, tag="stage")
                                for ui in range(2):
                                    nc.tensor.matmul(
                                        out=bank[:, 512 * ui:512 * (ui + 1)],
                                        lhsT=wts[layer][:, 128 * pi:128 * (pi + 1)],
                                        rhs=cur[ui][:, :],
                                        start=True, stop=True,
                                    )
                                h = actp.tile([128, 1024], BF16, tag="h")
                                bcol = 3 * layer + pi
                                if drain_ct % 9 < 2:
                                    nc.vector.tensor_scalar(
                                        out=h[:, :], in0=bank[:, :],
                                        scalar1=biast[:, bcol:bcol + 1], scalar2=None,
                                        op0=ALU.add)
                                    nc.vector.scalar_tensor_tensor(
                                        out=h[:, :], in0=h[:, :], scalar=NEG, in1=h[:, :],
                                        op0=ALU.mult, op1=ALU.max)
                                else:
                                    nc.scalar.activation(
                                        out=h[:, :], in_=bank[:, :], func=ACTF.Lrelu,
                                        bias=biast[:, bcol:bcol + 1], alpha=NEG)
                                drain_ct += 1
                                cur = [h[:, 0:512], h[:, 512:1024]]
                            for ui in range(2):
                                t_idx = 4 * b8 + 2 * up + ui
                                k3 = 128 * (32 * pi + t_idx)
                                nc.tensor.matmul(
                                    out=l3banks[pi][:, :],
                                    lhsT=w3t[:, k3:k3 + 128],
                                    rhs=cur[ui],
                                    start=(t_idx == 0), stop=(t_idx == 31),
                                    skip_group_check=True,
                                )
                # ---- tail ----
                preds = []
                for pi in range(3):
                    pclip = tailp.tile([128, 512], F32, tag="t%d" % pi)
                    nc.vector.tensor_scalar(
                        out=pclip[:, :], in0=l3banks[pi][:, :],
                        scalar1=float(b3[pi]), scalar2=1.0,
                        op0=ALU.add, op1=ALU.min)
                    nc.vector.tensor_scalar(
                        out=pclip[:, :], in0=pclip[:, :],
                        scalar1=-1.0, scalar2=None, op0=ALU.max)
                    preds.append(pclip)
                t1 = tailp.tile([128, 512], F32, tag="m1")
                t2 = tailp.tile([128, 512], F32, tag="m2")
                nc.vector.tensor_tensor(out=t1[:, :], in0=preds[1][:, :], in1=preds[2][:, :], op=ALU.max)
                nc.vector.tensor_tensor(out=t2[:, :], in0=preds[1][:, :], in1=preds[2][:, :], op=ALU.min)
                nc.vector.tensor_tensor(out=t1[:, :], in0=preds[0][:, :], in1=t1[:, :], op=ALU.min)
                nc.vector.tensor_tensor(out=t1[:, :], in0=t1[:, :], in1=t2[:, :], op=ALU.max)
                xt = tailp.tile([128, 512], F32, tag="xt")
                xsrc = bass.AP(tensor=xf32, offset=sl * 128 * 512,
                               ap=[[512, 128], [1, 512]])
                dma.dma_start(out=xt[:, :], in_=xsrc)
                y = tailp.tile([128, 512], F32, tag="y")
                nc.vector.tensor_tensor(out=y[:, :], in0=xt[:, :], in1=t1[:, :], op=ALU.subtract)
                kge = tailp.tile([128, 512], F32, tag="kg")
                nc.vector.tensor_scalar(out=kge[:, :], in0=y[:, :], scalar1=1.0,
                                        scalar2=None, op0=ALU.is_ge)
                delta = tailp.tile([128, 512], F32, tag="dl")
                nc.vector.scalar_tensor_tensor(
                    out=delta[:, :], in0=kge[:, :], scalar=-2.0, in1=y[:, :],
                    op0=ALU.mult, op1=ALU.add)
                scr = tailp.tile([128, 512], F32, tag="sc")
                nc.scalar.activation(out=scr[:, :], in_=delta[:, :], func=ACTF.Square,
                                     accum_out=ssq_acc[:, 2 * sl:2 * sl + 1])
                nc.scalar.activation(out=scr[:, :], in_=xt[:, :], func=ACTF.Square,
                                     accum_out=ssq_acc[:, 2 * sl + 1:2 * sl + 2])
                histbank = psH.tile([128, 256], F32, tag="hist")
                for hj, val in enumerate([xt, delta]):
                    z = tailp.tile([128, 512], F32, tag="z")
                    nc.vector.tensor_scalar(out=z[:, :], in0=val[:, :], scalar1=128.0,
                                            scalar2=256.0, op0=ALU.mult, op1=ALU.add)
                    f = tailp.tile([128, 512], F32, tag="f")
                    M23 = 8388608.0
                    nc.vector.tensor_scalar(out=f[:, :], in0=z[:, :], scalar1=M23,
                                            scalar2=M23, op0=ALU.add, op1=ALU.subtract)
                    g = tailp.tile([128, 512], F32, tag="g")
                    nc.vector.tensor_tensor(out=g[:, :], in0=f[:, :], in1=z[:, :], op=ALU.is_gt)
                    nc.vector.tensor_tensor(out=z[:, :], in0=f[:, :], in1=g[:, :], op=ALU.subtract)
                    hi = tailp.tile([128, 512], F32, tag="hi")
                    q = tailp.tile([128, 512], F32, tag="q")
                    nc.vector.tensor_scalar(out=q[:, :], in0=z[:, :], scalar1=0.0625,
                                            scalar2=None, op0=ALU.mult)
                    nc.vector.tensor_scalar(out=f[:, :], in0=q[:, :], scalar1=M23,
                                            scalar2=M23, op0=ALU.add, op1=ALU.subtract)
                    nc.vector.tensor_tensor(out=g[:, :], in0=f[:, :], in1=q[:, :], op=ALU.is_gt)
                    nc.vector.tensor_tensor(out=hi[:, :], in0=f[:, :], in1=g[:, :], op=ALU.subtract)
                    lo = tailp.tile([128, 512], F32, tag="lo")
                    nc.vector.scalar_tensor_tensor(
                        out=lo[:, :], in0=hi[:, :], scalar=-16.0, in1=z[:, :],
                        op0=ALU.mult, op1=ALU.add)
                    hi_bf = ohp.tile([128, 512], BF16, tag="hb")
                    lo_bf = ohp.tile([128, 512], BF16, tag="lb")
                    nc.vector.tensor_copy(out=hi_bf[:, :], in_=hi[:, :])
                    nc.vector.tensor_copy(out=lo_bf[:, :], in_=lo[:, :])
                    A = ohp.tile([128, 8192], BF16, tag="A")
                    B = ohp.tile([128, 8192], BF16, tag="B")
                    for blk in range(16):
                        c0 = 32 * blk
                        src_hi = bass.AP(tensor=hi_bf.tensor, offset=hi_bf.offset + c0,
                                         ap=[hi_bf.ap[0], [8, 4], [0, 16], [1, 8]])
                        src_lo = bass.AP(tensor=lo_bf.tensor, offset=lo_bf.offset + c0,
                                         ap=[lo_bf.ap[0], [8, 4], [0, 16], [1, 8]])
                        ioA = bass.AP(tensor=iotaAt.tensor, offset=iotaAt.offset,
                                      ap=[iotaAt.ap[0], [0, 4], [8, 16], [1, 8]])
                        ioB = bass.AP(tensor=iotaBt.tensor, offset=iotaBt.offset,
                                      ap=[iotaBt.ap[0], [0, 4], [8, 16], [1, 8]])
                        dstA = bass.AP(tensor=A.tensor, offset=A.offset + 512 * blk,
                                       ap=[A.ap[0], [128, 4], [8, 16], [1, 8]])
                        dstB = bass.AP(tensor=B.tensor, offset=B.offset + 512 * blk,
                                       ap=[B.ap[0], [128, 4], [8, 16], [1, 8]])
                        nc.vector.tensor_tensor(out=dstA, in0=src_hi, in1=ioA, op=ALU.is_equal)
                        nc.vector.tensor_tensor(out=dstB, in0=src_lo, in1=ioB, op=ALU.is_equal)
                    for m in range(64):
                        nc.tensor.matmul(
                            out=histbank[:, 128 * hj:128 * (hj + 1)],
                            lhsT=A[:, 128 * m:128 * (m + 1)],
                            rhs=B[:, 128 * m:128 * (m + 1)],
                            start=(m == 0), stop=(m == 63),
                            skip_group_check=True,
                        )
                hsb = tailp.tile([128, 256], F32, tag="hsb")
                nc.vector.tensor_copy(out=hsb[:, :], in_=histbank[:, :])
                dma.dma_start(out=hist_out[128 * sl:128 * (sl + 1), :], in_=hsb[:, :])
            dma.dma_start(out=ssq_out[:, :], in_=ssq_acc[:, :])
    nc.compile()
    return nc


def kernel(**inputs):
    x = np.asarray(inputs["x"], np.float32)  # [4,3,512,512]
    w0bd, w1bd, w2bd, w3z, biases, b3, iotaA, iotaB = _build_weight_arrays(inputs)
    key = b3.tobytes()
    if key not in _CACHE:
        _CACHE[key] = _build_bass(b3)
    nc = _CACHE[key]

    xr = x.reshape(12, 512, 512)
    in_maps = []
    for core in range(8):
        xp = np.zeros((NSLICE, ROWS + 3, WP), np.float32)
        for j in range(NSLICE):
            gsl = 6 * core + j
            ch, s4 = gsl // 4, gsl % 4
            r0 = 128 * s4
            lo_r = max(r0 - 3, 0)
            xp[j, 3 - (r0 - lo_r):3 + 128, 3:515] = xr[ch, lo_r:r0 + 128, :]
        xfc = np.zeros((NSLICE, 128, 512), np.float32)
        for j in range(NSLICE):
            gsl = 6 * core + j
            ch, s4 = gsl // 4, gsl % 4
            xfc[j] = xr[ch, 128 * s4:128 * s4 + 128, :]
        in_maps.append({
            "xpad": xp.reshape(-1).astype(ml_dtypes.bfloat16),
            "xf32": xfc.reshape(-1),
            "w0": w0bd.reshape(3 * 128, 128).astype(ml_dtypes.bfloat16),
            "w1": w1bd.reshape(3 * 128, 128).astype(ml_dtypes.bfloat16),
            "w2": w2bd.reshape(3 * 128, 128).astype(ml_dtypes.bfloat16),
            "w3": w3z.reshape(96 * 128, 128).astype(ml_dtypes.bfloat16),
            "bias_in": biases,
            "iotaA_in": iotaA,
            "iotaB_in": iotaB,
        })
    res = run_bass_kernel_spmd(nc, in_maps, core_ids=list(range(8)))
    global _LAST_RUN
    _LAST_RUN = res

    # ---- host epilogue: all-reduce partials + scalar math ----
    ssq_d = 0.0
    ssq_x = 0.0
    counts_x = np.zeros((12, 256), np.float64)
    counts_d = np.zeros((12, 256), np.float64)
    for core in range(8):
        out = res.results[core]
        ssq = np.asarray(out["ssq_out"], np.float64)  # [128, 12]
        hist = np.asarray(out["hist_out"], np.float64).reshape(NSLICE, 128, 256)
        for j in range(NSLICE):
            gsl = 6 * core + j
            ch = gsl // 4
            ssq_d += ssq[:, 2 * j].sum()
            ssq_x += ssq[:, 2 * j + 1].sum()
            raw = hist[j]  # [128, 256]; cols 0:128 = x, 128:256 = deltas
            for hj, tgt in enumerate([counts_x, counts_d]):
                blk = raw[:, 128 * hj:128 * (hj + 1)]
                for s in range(8):
                    tgt[ch] += blk[(8 * np.arange(16)[:, None] + s),
                                   (8 * np.arange(16)[None, :] + s)].reshape(256)

    npix = 12 * 512 * 512
    loss1 = np.float32(255.0 * np.sqrt(ssq_d / npix))
    loss0 = np.float32(255.0 * np.sqrt(ssq_x / npix))

    def inv_cr(counts):
        res_pix = 512 * 512
        p = counts / res_pix
        ent = -(p * np.log2(np.where(p > 0, p, 1.0))).sum()
        return np.float32(ent / (8.0 * 12))

    return (loss1, loss0, inv_cr(counts_x), inv_cr(counts_d))

